# revision 25
# baseline (speedup 1.0000x reference)
"""Causal self-attention (B=4, T=2048, C=1024, NH=16) on 8 TRN2 NeuronCores.

Sharding: tensor-parallel over heads - 2 heads per core. Each core computes
its slice of qkv (transposed layout), full causal attention for its heads,
and a partial output projection; the host sums the 8 bf16 partials and adds
b_proj.

All matmul operands are bf16 (fp32 PSUM accumulation). bf16 enables the
fast-weight-load path so LDWEIGHTS overlaps the matmul stream, and halves
DMA + DVE-copy traffic. Measured rel err stays ~1e-3, far under the 2e-2
budget.

Attention is emitted qc-major (one 512-wide q chunk at a time per batch):
for each k-chunk j the two heads' score matmuls write one [128,1024] PSUM
pair, a single fused Exp covers both heads, and pv accumulates into one
[65,512] PSUM bank per head (ones-column computes the softmax denominator).
The j-loop is software-pipelined as sc(j) / exp(j) / pv(j-1) so the PE
never sits on the ACT dependency, with qkv(b+1) / vT-transpose / proj(b-1)
units interleaved between attention units to keep the PE array dense (and
HAM-warm).

PSUM budget (8 banks): sc pair 2 + yts 2 + qkv fill 2 + proj 1 + vT 0.25.
"""

import sys

import numpy as np

try:
    import concourse.bass as bass
except ImportError:  # grading container may not have it on sys.path
    sys.path.insert(0, "/opt/trn_rl_repo")
    import concourse.bass as bass

from contextlib import ExitStack

import ml_dtypes
import concourse.mybir as mybir
import concourse.tile as tile
from concourse.bass_utils import run_bass_kernel_spmd


B, T, C, NH, HD = 4, 2048, 1024, 16, 64
N_CORES = 8
HPC = NH // N_CORES  # heads per core = 2
DPC = HPC * HD  # dims per core = 128
BT = B * T  # 8192
QCH = 512  # q-chunk
KCH = 128  # k-chunk
TCH = 512  # token chunk for qkv
NKC = C // 128  # 8 contraction chunks for qkv
NTC = T // TCH  # 4 token chunks per batch
NQC = T // QCH  # 4 q-chunks per batch
NVC = T // KCH  # 16 k/v chunks per batch
F32 = mybir.dt.float32
BF16 = mybir.dt.bfloat16
AF = mybir.ActivationFunctionType


def _split_multi_waits(nc):
    """Walrus in this container accepts only ONE sync wait per instruction.
    Hoist extra waits onto same-engine NoOps inserted just before."""
    n = 0
    for f in nc.m.functions:
        for b in f.blocks:
            insts = b.instructions
            if not any(
                i.sync_info is not None
                and i.sync_info.on_wait
                and len(i.sync_info.on_wait) > 1
                for i in insts
            ):
                continue
            new = []
            for ins in insts:
                si = ins.sync_info
                if si is not None and si.on_wait and len(si.on_wait) > 1:
                    waits = list(si.on_wait)
                    for w in waits[:-1]:
                        nop = mybir.InstNoOp(
                            name=f"{ins.name}-ws{n}", ins=[], outs=[]
                        )
                        nop.engine = ins.engine
                        nop.bass_nofuse = True
                        nop.sync_info = mybir.SyncInfo(on_wait=[w], on_update=[])
                        if ins.debug is not None:
                            nop.debug = ins.debug
                        new.append(nop)
                        n += 1
                    ins.sync_info = mybir.SyncInfo(
                        on_wait=[waits[-1]], on_update=list(si.on_update or [])
                    )
                new.append(ins)
            b.instructions = new
    return n


def build_kernel():
    nc = bass.Bass("TRN2", target_bir_lowering=False, debug=False, num_devices=N_CORES)
    xT_d = nc.dram_tensor("xT", [C, BT], BF16, kind="ExternalInput")
    wc_d = nc.dram_tensor("wc", [C, 3 * DPC], BF16, kind="ExternalInput")
    bc_d = nc.dram_tensor("bc", [3, DPC, 1], F32, kind="ExternalInput")
    wp_d = nc.dram_tensor("wp", [DPC, C], BF16, kind="ExternalInput")
    out_d = nc.dram_tensor("out", [BT, C], BF16, kind="ExternalOutput")
    # scratch for the softmax-reciprocal row: bouncing through DRAM lets
    # the broadcast read use a stride-0 row (illegal on an SBUF source)
    rc_d = nc.dram_tensor("rc_scratch", [B * NQC, 2 * QCH], BF16, kind="Internal")

    with tile.TileContext(nc) as tc, ExitStack() as ctx:
        consts = ctx.enter_context(tc.tile_pool(name="consts", bufs=1))
        xpool = ctx.enter_context(tc.tile_pool(name="x", bufs=16))
        qkvp = ctx.enter_context(tc.tile_pool(name="qkv", bufs=2))
        vexp = ctx.enter_context(tc.tile_pool(name="vext", bufs=2))
        ytp = ctx.enter_context(tc.tile_pool(name="yt", bufs=2))
        expp = ctx.enter_context(tc.tile_pool(name="expt", bufs=6))
        smallp = ctx.enter_context(tc.tile_pool(name="small", bufs=3))
        outp = ctx.enter_context(tc.tile_pool(name="outt", bufs=4))
        # PSUM: 8 banks total. sc 2 + yt 2 + qk 2 + po 1 + pt 0.25
        ps_sc = ctx.enter_context(tc.tile_pool(name="ps_sc", bufs=1, space="PSUM"))
        ps_yt = ctx.enter_context(tc.tile_pool(name="ps_yt", bufs=2, space="PSUM"))
        ps_qk = ctx.enter_context(tc.tile_pool(name="ps_qk", bufs=2, space="PSUM"))
        ps_po = ctx.enter_context(tc.tile_pool(name="ps_po", bufs=2, space="PSUM"))

        # [128,128] bf16 identity for PE transposes
        ident = consts.tile([128, 128], BF16)
        nc.gpsimd.memset(ident, 0.0)
        nc.gpsimd.affine_select(
            out=ident,
            in_=ident,
            compare_op=mybir.AluOpType.not_equal,
            fill=1.0,
            base=0,
            pattern=[[-1, 128]],
            channel_multiplier=1,
        )

        # weights: wc [1024, 384] -> [128, 8, 384] (kc chunks on free dim)
        w_sb = consts.tile([128, 8, 3 * DPC], BF16)
        nc.sync.dma_start(
            out=w_sb, in_=wc_d.ap().rearrange("(kc p) c -> p kc c", p=128)
        )
        wp_sb = consts.tile([128, C], BF16)
        nc.sync.dma_start(out=wp_sb, in_=wp_d.ap())
        bc_sb = consts.tile([128, 3], F32)
        nc.sync.dma_start(out=bc_sb, in_=bc_d.ap().rearrange("g p one -> p (g one)"))

        # -------- unit-based emission with explicit cross-phase interleave.
        state = {}

        def qkv_units(b):
            t0 = b * T
            st = state.setdefault(b, {})
            units = []

            def alloc(b=b, st=st):
                st["qT"] = qkvp.tile([128, T], BF16, name=f"qT_{b}", tag="qT")
                st["kT"] = qkvp.tile([128, T], BF16, name=f"kT_{b}", tag="kT")
                st["vT"] = qkvp.tile([128, T], BF16, name=f"vT_{b}", tag="vT")
                st["xts"] = {}

            units.append(alloc)
            for tcb in range(NTC):

                def dma_u(tcb=tcb, st=st, t0=t0, b=b):
                    xts = []
                    for kc in range(NKC):
                        xt = xpool.tile(
                            [128, TCH], BF16, name=f"xt_{b}_{tcb}_{kc}", tag="xt"
                        )
                        nc.sync.dma_start(
                            out=xt,
                            in_=xT_d.ap()[
                                kc * 128 : (kc + 1) * 128,
                                t0 + tcb * TCH : t0 + (tcb + 1) * TCH,
                            ],
                        )
                        xts.append(xt)
                    st["xts"][tcb] = xts

                units.append(dma_u)
                for g in range(3):
                    # split each accumulation group in two emission units so
                    # fills interleave at finer grain (same psum tile).
                    def mm_u1(tcb=tcb, g=g, st=st, b=b):
                        ps = ps_qk.tile(
                            [128, TCH], F32, name=f"qkvps_{b}_{tcb}_{g}", tag="qk"
                        )
                        st["qkv_ps"] = ps
                        for kc in range(4):
                            nc.tensor.matmul(
                                ps,
                                w_sb[:, kc, g * 128 : (g + 1) * 128],
                                st["xts"][tcb][kc],
                                start=(kc == 0),
                                stop=False,
                            )

                    def mm_u2(tcb=tcb, g=g, st=st, b=b):
                        ps = st["qkv_ps"]
                        dest = [st["qT"], st["kT"], st["vT"]]
                        for kc in range(4, NKC):
                            nc.tensor.matmul(
                                ps,
                                w_sb[:, kc, g * 128 : (g + 1) * 128],
                                st["xts"][tcb][kc],
                                start=False,
                                stop=(kc == NKC - 1),
                            )
                        # psum -> sbuf bf16 with bias add on DVE
                        nc.vector.tensor_scalar_add(
                            dest[g][:, tcb * TCH : (tcb + 1) * TCH],
                            ps,
                            bc_sb[:, g : g + 1],
                        )

                    units.append(mm_u1)
                    units.append(mm_u2)
            return units

        def vt_units(b):
            st = state.setdefault(b, {})
            units = []

            def alloc(st=st, b=b):
                st["vex"] = vexp.tile(
                    [128, NVC, HPC, 65], BF16, name=f"vex_{b}", tag="vex"
                )
                nc.vector.memset(st["vex"][:, :, :, 64:65], 1.0)

            units.append(alloc)
            for j in range(NVC):

                def tr_u(j=j, st=st, b=b):
                    # transpose both heads' [128d, 128t] chunk in one PE shot
                    # (sharing the proj psum ring), then one fused DVE copy
                    ptf = ps_po.tile([128, 512], F32, name=f"vtps_{b}_{j}", tag="po")
                    pt = ptf[:, 0:64].bitcast(BF16)  # [128, 128] bf16 view
                    nc.tensor.transpose(
                        pt, st["vT"][:, j * 128 : (j + 1) * 128], ident
                    )
                    nc.vector.tensor_copy(
                        st["vex"][:, j, :, 0:64],
                        pt.rearrange("p (h d) -> p h d", h=HPC),
                    )

                units.append(tr_u)
            return units

        def attn_units(b, proj_tail=None):
            """proj_tail: optional per-qc list of extra units (e.g. proj of
            this batch's finished q-chunks) appended after each qc's norm."""
            st = state[b]
            units = []

            def alloc_yt(st=st, b=b):
                st["yT"] = ytp.tile([128, T], BF16, name=f"yT_{b}", tag="yT")

            units.append(alloc_yt)
            pending = []  # delayed units (norm of previous qc) to interleave
            for qc in range(NQC):
                nj = 4 * (qc + 1)  # k-chunks for this q chunk

                def alloc_chains(qc=qc, st=st, b=b):
                    st[("yts", qc)] = [
                        ps_yt.tile(
                            [65, QCH], F32, name=f"yt_{b}_{qc}_{h}", tag="yt"
                        )
                        for h in range(HPC)
                    ]

                def sc_u(j, qc=qc, st=st, b=b):
                    q0 = qc * QCH
                    k0 = j * KCH
                    qlo = max(0, k0 - q0)
                    sc = ps_sc.tile(
                        [128, 2 * QCH], F32, name=f"sc_{b}_{qc}_{j}", tag="sc"
                    )
                    st[("sc", j)] = (sc, qlo)
                    for h in range(HPC):
                        nc.tensor.matmul(
                            sc[:, h * QCH + qlo : (h + 1) * QCH],
                            st["kT"][64 * h : 64 * h + 64, k0 : k0 + KCH],
                            st["qT"][64 * h : 64 * h + 64, q0 + qlo : q0 + QCH],
                            start=True,
                            stop=True,
                        )

                def exp_u(j, qc=qc, st=st, b=b):
                    sc, qlo = st[("sc", j)]
                    ex = expp.tile(
                        [128, 2 * QCH], BF16, name=f"ex_{b}_{qc}_{j}", tag="ex"
                    )
                    st[("ex", j)] = (ex, qlo)
                    if qlo == 0:
                        # one fused exp across both heads
                        nc.scalar.activation(
                            ex[:, 0 : 2 * QCH], sc[:, 0 : 2 * QCH], AF.Exp,
                            scale=0.125,
                        )
                    else:
                        # diagonal block: skip the stale gap between the two
                        # heads' valid column ranges
                        for h in range(HPC):
                            nc.scalar.activation(
                                ex[:, h * QCH + qlo : (h + 1) * QCH],
                                sc[:, h * QCH + qlo : (h + 1) * QCH],
                                AF.Exp,
                                scale=0.125,
                            )
                    if j * KCH >= qc * QCH:
                        # diagonal block: zero where k > q, per head
                        for h in range(HPC):
                            nc.gpsimd.affine_select(
                                out=ex[:, h * QCH + qlo : h * QCH + qlo + 128],
                                in_=ex[:, h * QCH + qlo : h * QCH + qlo + 128],
                                compare_op=mybir.AluOpType.is_ge,
                                fill=0.0,
                                base=0,
                                pattern=[[1, 128]],
                                channel_multiplier=-1,
                            )

                def pv_u(j, qc=qc, nj=nj, st=st, b=b):
                    ex, qlo = st[("ex", j)]
                    for h in range(HPC):
                        nc.tensor.matmul(
                            st[("yts", qc)][h][:, qlo:QCH],
                            st["vex"][:, j, h, :],
                            ex[:, h * QCH + qlo : (h + 1) * QCH],
                            start=(j == 0),
                            stop=(j == nj - 1),
                        )

                def norm_u1(qc=qc, st=st, b=b):
                    # 1/denominator = exp(-ln(s)); Ln+Exp share one ACT
                    # table set, and reading sums via ACT-Ln from PSUM
                    # skips a DVE gather.
                    sums = smallp.tile([1, 2 * QCH], F32, name=f"s_{b}_{qc}", tag="s")
                    st[("sums", qc)] = sums
                    for h in range(HPC):
                        nc.scalar.activation(
                            sums[:, h * QCH : (h + 1) * QCH],
                            st[("yts", qc)][h][64:65, :],
                            AF.Ln,
                        )

                def norm_u2(qc=qc, st=st, b=b):
                    recip = smallp.tile(
                        [1, 2 * QCH], BF16, name=f"rc_{b}_{qc}", tag="rc"
                    )
                    nc.scalar.activation(recip, st[("sums", qc)], AF.Exp, scale=-1.0)
                    nc.sync.dma_start(
                        out=rc_d.ap()[b * NQC + qc : b * NQC + qc + 1, :], in_=recip
                    )

                def norm_u3(h, qc=qc, st=st, b=b):
                    # broadcast recip row to 64 partitions on the DMA engine
                    bch = smallp.tile(
                        [64, QCH], BF16, name=f"bc_{b}_{qc}_{h}", tag=f"bc{h}"
                    )
                    nc.sync.dma_start(
                        out=bch,
                        in_=rc_d.ap()[
                            b * NQC + qc : b * NQC + qc + 1,
                            h * QCH : (h + 1) * QCH,
                        ].to_broadcast((64, QCH)),
                    )
                    nc.vector.tensor_mul(
                        st["yT"][64 * h : 64 * h + 64, qc * QCH : (qc + 1) * QCH],
                        st[("yts", qc)][h][0:64, :],
                        bch,
                    )

                # software pipeline: sc(j), exp(j), pv(j-1); the previous
                # qc's norm chain (and any ready proj-tail units) is spliced
                # in after sc(0)/exp(0) so the PE keeps streaming while
                # Ln/Exp/broadcast run. alloc_chains must follow the norm
                # units (its ring slots are the previous qc's yts) but
                # precede pv(0).
                for j in range(nj):
                    units.append(lambda j=j, f=sc_u: f(j))
                    units.append(lambda j=j, f=exp_u: f(j))
                    if j == 0:
                        units.extend(pending)
                        pending = []
                        units.append(alloc_chains)
                    if j > 0:
                        units.append(lambda j=j, f=pv_u: f(j - 1))
                units.append(lambda nj=nj, f=pv_u: f(nj - 1))

                pending = [
                    norm_u1,
                    norm_u2,
                    lambda f=norm_u3: f(0),
                    lambda f=norm_u3: f(1),
                ]
                if proj_tail is not None:
                    pending = pending + proj_tail[qc]
            units.extend(pending)
            return units

        def proj_units(b, extra_pool=False, by_qc=False):
            st = state[b]
            t0 = b * T
            units = []
            for tcb in range(T // 128):
                for g in range(2):

                    def p_u(tcb=tcb, g=g, st=st, t0=t0, b=b):
                        pool = ps_qk if (extra_pool and g == 1) else ps_po
                        tag = "qk" if (extra_pool and g == 1) else "po"
                        ps = pool.tile(
                            [128, 512], F32, name=f"pps_{b}_{tcb}_{g}", tag=tag
                        )
                        nc.tensor.matmul(
                            ps,
                            st["yT"][:, tcb * 128 : (tcb + 1) * 128],
                            wp_sb[:, g * 512 : (g + 1) * 512],
                            start=True,
                            stop=True,
                        )
                        ot = outp.tile(
                            [128, 512], BF16, name=f"ot_{b}_{tcb}_{g}", tag="ot"
                        )
                        nc.vector.tensor_copy(ot, ps)
                        nc.sync.dma_start(
                            out=out_d.ap()[
                                t0 + tcb * 128 : t0 + (tcb + 1) * 128,
                                g * 512 : (g + 1) * 512,
                            ],
                            in_=ot,
                        )

                    units.append(p_u)
            if by_qc:  # group by the q-chunk whose yT cols they read
                return [units[8 * qc : 8 * qc + 8] for qc in range(NQC)]
            return units

        def interleave(main, fill):
            """emit main units with fill units spread evenly between them"""
            out = []
            nf, nm = len(fill), len(main)
            fi = 0
            for mi, m in enumerate(main):
                out.append(m)
                want = (mi + 1) * nf // nm
                while fi < want:
                    out.append(fill[fi])
                    fi += 1
            out.extend(fill[fi:])
            return out

        for u in qkv_units(0) + vt_units(0):
            u()
        for b in range(B):
            tail = (
                proj_units(b, extra_pool=True, by_qc=True) if b == B - 1 else None
            )
            main = attn_units(b, proj_tail=tail)
            fill = []
            if b + 1 < B:
                fill += qkv_units(b + 1) + vt_units(b + 1)
            if b >= 1:
                fill += proj_units(b - 1, extra_pool=(b - 1 == 2))
            for u in interleave(main, fill):
                u()

    _split_multi_waits(nc)
    return nc


_NC_CACHE = None


def _get_nc():
    global _NC_CACHE
    if _NC_CACHE is None:
        _NC_CACHE = build_kernel()
    return _NC_CACHE


def kernel_with_results(x, W_attn, b_attn, W_proj, b_proj, trace=False):
    bf = ml_dtypes.bfloat16
    x = np.asarray(x, dtype=np.float32)
    W_attn = np.asarray(W_attn, dtype=np.float32)
    b_attn = np.asarray(b_attn, dtype=np.float32)
    W_proj = np.asarray(W_proj, dtype=np.float32)
    b_proj = np.asarray(b_proj, dtype=np.float32)

    xT = np.ascontiguousarray(x.reshape(BT, C).T).astype(bf)  # [C, BT]
    in_maps = []
    for c in range(N_CORES):
        lo = c * DPC
        wc = np.ascontiguousarray(
            np.concatenate(
                [
                    W_attn[:, lo : lo + DPC],
                    W_attn[:, C + lo : C + lo + DPC],
                    W_attn[:, 2 * C + lo : 2 * C + lo + DPC],
                ],
                axis=1,
            )
        ).astype(bf)
        bc = np.ascontiguousarray(
            np.stack(
                [
                    b_attn[lo : lo + DPC],
                    b_attn[C + lo : C + lo + DPC],
                    b_attn[2 * C + lo : 2 * C + lo + DPC],
                ]
            ).reshape(3, DPC, 1)
        )
        wp = np.ascontiguousarray(W_proj[lo : lo + DPC, :]).astype(bf)
        in_maps.append({"xT": xT, "wc": wc, "bc": bc, "wp": wp})

    nc = _get_nc()
    res = run_bass_kernel_spmd(
        nc, in_maps, core_ids=list(range(N_CORES)), trace=trace
    )
    acc = np.zeros((BT, C), dtype=np.float32)
    for c in range(N_CORES):
        acc += res.results[c]["out"].astype(np.float32)
    out = acc + b_proj
    return out.reshape(B, T, C), res


def kernel(x, W_attn, b_attn, W_proj, b_proj):
    out, _ = kernel_with_results(x, W_attn, b_attn, W_proj, b_proj)
    return out


if __name__ == "__main__":
    import jax

    key = jax.random.key(0)
    ks = jax.random.split(key, 5)
    import jax.numpy as jnp

    inputs = {
        "x": jax.random.normal(ks[0], (B, T, C), dtype=jnp.float32),
        "W_attn": jax.random.normal(ks[1], (C, 3 * C), dtype=jnp.float32) * 0.02,
        "b_attn": jnp.zeros((3 * C,), dtype=jnp.float32),
        "W_proj": jax.random.normal(ks[2], (C, C), dtype=jnp.float32) * 0.02,
        "b_proj": jnp.zeros((C,), dtype=jnp.float32),
    }
    out = kernel(**{k: np.asarray(v) for k, v in inputs.items()})
    print(out.shape, out.dtype)


# revision 27
# speedup vs baseline: 1.0347x; 1.0347x over previous
"""Causal self-attention (B=4, T=2048, C=1024, NH=16) on 8 TRN2 NeuronCores.

Sharding: tensor-parallel over heads - 2 heads per core. Each core computes
its slice of qkv (transposed layout), full causal attention for its heads,
and a partial output projection; the host sums the 8 bf16 partials and adds
b_proj.

All matmul operands are bf16 (fp32 PSUM accumulation). bf16 enables the
fast-weight-load path so LDWEIGHTS overlaps the matmul stream, and halves
DMA + DVE-copy traffic. Measured rel err stays ~1e-3, far under the 2e-2
budget.

Attention is emitted qc-major (one 512-wide q chunk at a time per batch):
for each k-chunk j the two heads' score matmuls write one [128,1024] PSUM
pair, a single fused Exp covers both heads, and pv accumulates into one
[65,512] PSUM bank per head (ones-column computes the softmax denominator).
The j-loop is software-pipelined as sc(j) / exp(j) / pv(j-1) so the PE
never sits on the ACT dependency, with qkv(b+1) / vT-transpose / proj(b-1)
units interleaved between attention units to keep the PE array dense (and
HAM-warm).

PSUM budget (8 banks): sc pair 2 + yts 2 + qkv fill 2 + proj 1 + vT 0.25.
"""

import sys

import numpy as np

try:
    import concourse.bass as bass
except ImportError:  # grading container may not have it on sys.path
    sys.path.insert(0, "/opt/trn_rl_repo")
    import concourse.bass as bass

from contextlib import ExitStack

import ml_dtypes
import concourse.mybir as mybir
import concourse.tile as tile
from concourse.bass_utils import run_bass_kernel_spmd


B, T, C, NH, HD = 4, 2048, 1024, 16, 64
N_CORES = 8
HPC = NH // N_CORES  # heads per core = 2
DPC = HPC * HD  # dims per core = 128
BT = B * T  # 8192
QCH = 512  # q-chunk
KCH = 128  # k-chunk
TCH = 512  # token chunk for qkv
NKC = C // 128  # 8 contraction chunks for qkv
NTC = T // TCH  # 4 token chunks per batch
NQC = T // QCH  # 4 q-chunks per batch
NVC = T // KCH  # 16 k/v chunks per batch
F32 = mybir.dt.float32
BF16 = mybir.dt.bfloat16
AF = mybir.ActivationFunctionType


def _split_multi_waits(nc):
    """Walrus in this container accepts only ONE sync wait per instruction.
    Hoist extra waits onto same-engine NoOps inserted just before."""
    n = 0
    for f in nc.m.functions:
        for b in f.blocks:
            insts = b.instructions
            if not any(
                i.sync_info is not None
                and i.sync_info.on_wait
                and len(i.sync_info.on_wait) > 1
                for i in insts
            ):
                continue
            new = []
            for ins in insts:
                si = ins.sync_info
                if si is not None and si.on_wait and len(si.on_wait) > 1:
                    waits = list(si.on_wait)
                    for w in waits[:-1]:
                        nop = mybir.InstNoOp(
                            name=f"{ins.name}-ws{n}", ins=[], outs=[]
                        )
                        nop.engine = ins.engine
                        nop.bass_nofuse = True
                        nop.sync_info = mybir.SyncInfo(on_wait=[w], on_update=[])
                        if ins.debug is not None:
                            nop.debug = ins.debug
                        new.append(nop)
                        n += 1
                    ins.sync_info = mybir.SyncInfo(
                        on_wait=[waits[-1]], on_update=list(si.on_update or [])
                    )
                new.append(ins)
            b.instructions = new
    return n


def build_kernel():
    nc = bass.Bass("TRN2", target_bir_lowering=False, debug=False, num_devices=N_CORES)
    xT_d = nc.dram_tensor("xT", [C, BT], BF16, kind="ExternalInput")
    wc_d = nc.dram_tensor("wc", [C, 3 * DPC], BF16, kind="ExternalInput")
    bc_d = nc.dram_tensor("bc", [3, DPC, 1], F32, kind="ExternalInput")
    wp_d = nc.dram_tensor("wp", [DPC, C], BF16, kind="ExternalInput")
    out_d = nc.dram_tensor("out", [BT, C], BF16, kind="ExternalOutput")
    # scratch for the softmax-reciprocal row: bouncing through DRAM lets
    # the broadcast read use a stride-0 row (illegal on an SBUF source)
    rc_d = nc.dram_tensor("rc_scratch", [B * NQC, 2 * QCH], BF16, kind="Internal")

    with tile.TileContext(nc) as tc, ExitStack() as ctx:
        consts = ctx.enter_context(tc.tile_pool(name="consts", bufs=1))
        xpool = ctx.enter_context(tc.tile_pool(name="x", bufs=16))
        qkvp = ctx.enter_context(tc.tile_pool(name="qkv", bufs=2))
        vexp = ctx.enter_context(tc.tile_pool(name="vext", bufs=2))
        ytp = ctx.enter_context(tc.tile_pool(name="yt", bufs=2))
        expp = ctx.enter_context(tc.tile_pool(name="expt", bufs=6))
        smallp = ctx.enter_context(tc.tile_pool(name="small", bufs=3))
        outp = ctx.enter_context(tc.tile_pool(name="outt", bufs=4))
        # PSUM: 8 banks total. sc 2 + yt 2 + qk 2 + po 1 + pt 0.25
        ps_sc = ctx.enter_context(tc.tile_pool(name="ps_sc", bufs=2, space="PSUM"))
        ps_yt = ctx.enter_context(tc.tile_pool(name="ps_yt", bufs=2, space="PSUM"))
        ps_qk = ctx.enter_context(tc.tile_pool(name="ps_qk", bufs=1, space="PSUM"))
        ps_po = ctx.enter_context(tc.tile_pool(name="ps_po", bufs=1, space="PSUM"))

        # [128,128] bf16 identity for PE transposes
        ident = consts.tile([128, 128], BF16)
        nc.gpsimd.memset(ident, 0.0)
        nc.gpsimd.affine_select(
            out=ident,
            in_=ident,
            compare_op=mybir.AluOpType.not_equal,
            fill=1.0,
            base=0,
            pattern=[[-1, 128]],
            channel_multiplier=1,
        )

        # weights: wc [1024, 384] -> [128, 8, 384] (kc chunks on free dim)
        w_sb = consts.tile([128, 8, 3 * DPC], BF16)
        nc.sync.dma_start(
            out=w_sb, in_=wc_d.ap().rearrange("(kc p) c -> p kc c", p=128)
        )
        wp_sb = consts.tile([128, C], BF16)
        nc.sync.dma_start(out=wp_sb, in_=wp_d.ap())
        bc_sb = consts.tile([128, 3], F32)
        nc.sync.dma_start(out=bc_sb, in_=bc_d.ap().rearrange("g p one -> p (g one)"))

        # -------- unit-based emission with explicit cross-phase interleave.
        state = {}

        def qkv_units(b):
            t0 = b * T
            st = state.setdefault(b, {})
            units = []

            def alloc(b=b, st=st):
                st["qT"] = qkvp.tile([128, T], BF16, name=f"qT_{b}", tag="qT")
                st["kT"] = qkvp.tile([128, T], BF16, name=f"kT_{b}", tag="kT")
                st["vT"] = qkvp.tile([128, T], BF16, name=f"vT_{b}", tag="vT")
                st["xts"] = {}

            units.append(alloc)
            for tcb in range(NTC):

                def dma_u(tcb=tcb, st=st, t0=t0, b=b):
                    xts = []
                    for kc in range(NKC):
                        xt = xpool.tile(
                            [128, TCH], BF16, name=f"xt_{b}_{tcb}_{kc}", tag="xt"
                        )
                        nc.sync.dma_start(
                            out=xt,
                            in_=xT_d.ap()[
                                kc * 128 : (kc + 1) * 128,
                                t0 + tcb * TCH : t0 + (tcb + 1) * TCH,
                            ],
                        )
                        xts.append(xt)
                    st["xts"][tcb] = xts

                units.append(dma_u)
                for g in range(3):
                    # split each accumulation group in two emission units so
                    # fills interleave at finer grain (same psum tile).
                    def mm_u1(tcb=tcb, g=g, st=st, b=b):
                        ps = ps_qk.tile(
                            [128, TCH], F32, name=f"qkvps_{b}_{tcb}_{g}", tag="qk"
                        )
                        st["qkv_ps"] = ps
                        for kc in range(4):
                            nc.tensor.matmul(
                                ps,
                                w_sb[:, kc, g * 128 : (g + 1) * 128],
                                st["xts"][tcb][kc],
                                start=(kc == 0),
                                stop=False,
                            )

                    def mm_u2(tcb=tcb, g=g, st=st, b=b):
                        ps = st["qkv_ps"]
                        dest = [st["qT"], st["kT"], st["vT"]]
                        for kc in range(4, NKC):
                            nc.tensor.matmul(
                                ps,
                                w_sb[:, kc, g * 128 : (g + 1) * 128],
                                st["xts"][tcb][kc],
                                start=False,
                                stop=(kc == NKC - 1),
                            )
                        # psum -> sbuf bf16 with bias add on DVE
                        nc.vector.tensor_scalar_add(
                            dest[g][:, tcb * TCH : (tcb + 1) * TCH],
                            ps,
                            bc_sb[:, g : g + 1],
                        )

                    units.append(mm_u1)
                    units.append(mm_u2)
            return units

        def vt_units(b):
            st = state.setdefault(b, {})
            units = []

            def alloc(st=st, b=b):
                st["vex"] = vexp.tile(
                    [128, NVC, HPC, 65], BF16, name=f"vex_{b}", tag="vex"
                )
                nc.vector.memset(st["vex"][:, :, :, 64:65], 1.0)

            units.append(alloc)
            for j in range(NVC):

                def tr_u(j=j, st=st, b=b):
                    # transpose both heads' [128d, 128t] chunk in one PE shot
                    # (sharing the proj psum ring), then one fused DVE copy
                    ptf = ps_po.tile([128, 512], F32, name=f"vtps_{b}_{j}", tag="po")
                    pt = ptf[:, 0:64].bitcast(BF16)  # [128, 128] bf16 view
                    nc.tensor.transpose(
                        pt, st["vT"][:, j * 128 : (j + 1) * 128], ident
                    )
                    nc.vector.tensor_copy(
                        st["vex"][:, j, :, 0:64],
                        pt.rearrange("p (h d) -> p h d", h=HPC),
                    )

                units.append(tr_u)
            return units

        def attn_units(b, proj_tail=None):
            """proj_tail: optional per-qc list of extra units (e.g. proj of
            this batch's finished q-chunks) appended after each qc's norm."""
            st = state[b]
            units = []

            def alloc_yt(st=st, b=b):
                st["yT"] = ytp.tile([128, T], BF16, name=f"yT_{b}", tag="yT")

            units.append(alloc_yt)
            pending = []  # delayed units (norm of previous qc) to interleave
            for qc in range(NQC):
                nj = 4 * (qc + 1)  # k-chunks for this q chunk

                def alloc_chains(qc=qc, st=st, b=b):
                    st[("yts", qc)] = [
                        ps_yt.tile(
                            [65, QCH], F32, name=f"yt_{b}_{qc}_{h}", tag="yt"
                        )
                        for h in range(HPC)
                    ]

                def sc_u(j, qc=qc, st=st, b=b):
                    q0 = qc * QCH
                    k0 = j * KCH
                    qlo = max(0, k0 - q0)
                    sc = ps_sc.tile(
                        [128, 2 * QCH], F32, name=f"sc_{b}_{qc}_{j}", tag="sc"
                    )
                    st[("sc", j)] = (sc, qlo)
                    # 4-way 64x64 array tiling: (head h -> row group 64h,
                    # key-half kh -> col group 64kh) are pairwise-disjoint
                    # sub-rectangles of the PE array, so all four matmuls
                    # stream concurrently.
                    for h in range(HPC):
                        for kh in range(2):
                            nc.tensor.matmul(
                                sc[64 * kh : 64 * kh + 64, h * QCH + qlo : (h + 1) * QCH],
                                st["kT"][
                                    64 * h : 64 * h + 64,
                                    k0 + 64 * kh : k0 + 64 * kh + 64,
                                ],
                                st["qT"][64 * h : 64 * h + 64, q0 + qlo : q0 + QCH],
                                start=True,
                                stop=True,
                            )

                def exp_u(j, qc=qc, st=st, b=b):
                    sc, qlo = st[("sc", j)]
                    ex = expp.tile(
                        [128, 2 * QCH], BF16, name=f"ex_{b}_{qc}_{j}", tag="ex"
                    )
                    st[("ex", j)] = (ex, qlo)
                    if qlo == 0:
                        # one fused exp across both heads
                        nc.scalar.activation(
                            ex[:, 0 : 2 * QCH], sc[:, 0 : 2 * QCH], AF.Exp,
                            scale=0.125,
                        )
                    else:
                        # diagonal block: skip the stale gap between the two
                        # heads' valid column ranges
                        for h in range(HPC):
                            nc.scalar.activation(
                                ex[:, h * QCH + qlo : (h + 1) * QCH],
                                sc[:, h * QCH + qlo : (h + 1) * QCH],
                                AF.Exp,
                                scale=0.125,
                            )
                    if j * KCH >= qc * QCH:
                        # diagonal block: zero where k > q, per head
                        for h in range(HPC):
                            nc.gpsimd.affine_select(
                                out=ex[:, h * QCH + qlo : h * QCH + qlo + 128],
                                in_=ex[:, h * QCH + qlo : h * QCH + qlo + 128],
                                compare_op=mybir.AluOpType.is_ge,
                                fill=0.0,
                                base=0,
                                pattern=[[1, 128]],
                                channel_multiplier=-1,
                            )

                def pv_u(j, qc=qc, nj=nj, st=st, b=b):
                    ex, qlo = st[("ex", j)]
                    for h in range(HPC):
                        nc.tensor.matmul(
                            st[("yts", qc)][h][:, qlo:QCH],
                            st["vex"][:, j, h, :],
                            ex[:, h * QCH + qlo : (h + 1) * QCH],
                            start=(j == 0),
                            stop=(j == nj - 1),
                        )

                def norm_u1(qc=qc, st=st, b=b):
                    # 1/denominator = exp(-ln(s)); Ln+Exp share one ACT
                    # table set, and reading sums via ACT-Ln from PSUM
                    # skips a DVE gather.
                    sums = smallp.tile([1, 2 * QCH], F32, name=f"s_{b}_{qc}", tag="s")
                    st[("sums", qc)] = sums
                    for h in range(HPC):
                        nc.scalar.activation(
                            sums[:, h * QCH : (h + 1) * QCH],
                            st[("yts", qc)][h][64:65, :],
                            AF.Ln,
                        )

                def norm_u2(qc=qc, st=st, b=b):
                    recip = smallp.tile(
                        [1, 2 * QCH], BF16, name=f"rc_{b}_{qc}", tag="rc"
                    )
                    nc.scalar.activation(recip, st[("sums", qc)], AF.Exp, scale=-1.0)
                    nc.sync.dma_start(
                        out=rc_d.ap()[b * NQC + qc : b * NQC + qc + 1, :], in_=recip
                    )

                def norm_u3(h, qc=qc, st=st, b=b):
                    # broadcast recip row to 64 partitions on the DMA engine
                    bch = smallp.tile(
                        [64, QCH], BF16, name=f"bc_{b}_{qc}_{h}", tag=f"bc{h}"
                    )
                    nc.sync.dma_start(
                        out=bch,
                        in_=rc_d.ap()[
                            b * NQC + qc : b * NQC + qc + 1,
                            h * QCH : (h + 1) * QCH,
                        ].to_broadcast((64, QCH)),
                    )
                    nc.vector.tensor_mul(
                        st["yT"][64 * h : 64 * h + 64, qc * QCH : (qc + 1) * QCH],
                        st[("yts", qc)][h][0:64, :],
                        bch,
                    )

                # software pipeline: sc(j), exp(j), pv(j-1); the previous
                # qc's norm chain (and any ready proj-tail units) is spliced
                # in after sc(0)/exp(0) so the PE keeps streaming while
                # Ln/Exp/broadcast run. alloc_chains must follow the norm
                # units (its ring slots are the previous qc's yts) but
                # precede pv(0).
                for j in range(nj):
                    units.append(lambda j=j, f=sc_u: f(j))
                    units.append(lambda j=j, f=exp_u: f(j))
                    if j == 0:
                        units.extend(pending)
                        pending = []
                        units.append(alloc_chains)
                    if j > 0:
                        units.append(lambda j=j, f=pv_u: f(j - 1))
                units.append(lambda nj=nj, f=pv_u: f(nj - 1))

                pending = [
                    norm_u1,
                    norm_u2,
                    lambda f=norm_u3: f(0),
                    lambda f=norm_u3: f(1),
                ]
                if proj_tail is not None:
                    pending = pending + proj_tail[qc]
            units.extend(pending)
            return units

        def proj_units(b, extra_pool=False, by_qc=False):
            st = state[b]
            t0 = b * T
            units = []
            for tcb in range(T // 128):
                for g in range(2):

                    def p_u(tcb=tcb, g=g, st=st, t0=t0, b=b):
                        pool = ps_qk if (extra_pool and g == 1) else ps_po
                        tag = "qk" if (extra_pool and g == 1) else "po"
                        ps = pool.tile(
                            [128, 512], F32, name=f"pps_{b}_{tcb}_{g}", tag=tag
                        )
                        nc.tensor.matmul(
                            ps,
                            st["yT"][:, tcb * 128 : (tcb + 1) * 128],
                            wp_sb[:, g * 512 : (g + 1) * 512],
                            start=True,
                            stop=True,
                        )
                        ot = outp.tile(
                            [128, 512], BF16, name=f"ot_{b}_{tcb}_{g}", tag="ot"
                        )
                        nc.vector.tensor_copy(ot, ps)
                        nc.sync.dma_start(
                            out=out_d.ap()[
                                t0 + tcb * 128 : t0 + (tcb + 1) * 128,
                                g * 512 : (g + 1) * 512,
                            ],
                            in_=ot,
                        )

                    units.append(p_u)
            if by_qc:  # group by the q-chunk whose yT cols they read
                return [units[8 * qc : 8 * qc + 8] for qc in range(NQC)]
            return units

        def interleave(main, fill):
            """emit main units with fill units spread evenly between them"""
            out = []
            nf, nm = len(fill), len(main)
            fi = 0
            for mi, m in enumerate(main):
                out.append(m)
                want = (mi + 1) * nf // nm
                while fi < want:
                    out.append(fill[fi])
                    fi += 1
            out.extend(fill[fi:])
            return out

        for u in qkv_units(0) + vt_units(0):
            u()
        for b in range(B):
            tail = (
                proj_units(b, extra_pool=True, by_qc=True) if b == B - 1 else None
            )
            main = attn_units(b, proj_tail=tail)
            fill = []
            if b + 1 < B:
                fill += qkv_units(b + 1) + vt_units(b + 1)
            if b >= 1:
                fill += proj_units(b - 1, extra_pool=(b - 1 == 2))
            for u in interleave(main, fill):
                u()

    _split_multi_waits(nc)
    return nc


_NC_CACHE = None


def _get_nc():
    global _NC_CACHE
    if _NC_CACHE is None:
        _NC_CACHE = build_kernel()
    return _NC_CACHE


def kernel_with_results(x, W_attn, b_attn, W_proj, b_proj, trace=False):
    bf = ml_dtypes.bfloat16
    x = np.asarray(x, dtype=np.float32)
    W_attn = np.asarray(W_attn, dtype=np.float32)
    b_attn = np.asarray(b_attn, dtype=np.float32)
    W_proj = np.asarray(W_proj, dtype=np.float32)
    b_proj = np.asarray(b_proj, dtype=np.float32)

    xT = np.ascontiguousarray(x.reshape(BT, C).T).astype(bf)  # [C, BT]
    in_maps = []
    for c in range(N_CORES):
        lo = c * DPC
        wc = np.ascontiguousarray(
            np.concatenate(
                [
                    W_attn[:, lo : lo + DPC],
                    W_attn[:, C + lo : C + lo + DPC],
                    W_attn[:, 2 * C + lo : 2 * C + lo + DPC],
                ],
                axis=1,
            )
        ).astype(bf)
        bc = np.ascontiguousarray(
            np.stack(
                [
                    b_attn[lo : lo + DPC],
                    b_attn[C + lo : C + lo + DPC],
                    b_attn[2 * C + lo : 2 * C + lo + DPC],
                ]
            ).reshape(3, DPC, 1)
        )
        wp = np.ascontiguousarray(W_proj[lo : lo + DPC, :]).astype(bf)
        in_maps.append({"xT": xT, "wc": wc, "bc": bc, "wp": wp})

    nc = _get_nc()
    res = run_bass_kernel_spmd(
        nc, in_maps, core_ids=list(range(N_CORES)), trace=trace
    )
    acc = np.zeros((BT, C), dtype=np.float32)
    for c in range(N_CORES):
        acc += res.results[c]["out"].astype(np.float32)
    out = acc + b_proj
    return out.reshape(B, T, C), res


def kernel(x, W_attn, b_attn, W_proj, b_proj):
    out, _ = kernel_with_results(x, W_attn, b_attn, W_proj, b_proj)
    return out


if __name__ == "__main__":
    import jax

    key = jax.random.key(0)
    ks = jax.random.split(key, 5)
    import jax.numpy as jnp

    inputs = {
        "x": jax.random.normal(ks[0], (B, T, C), dtype=jnp.float32),
        "W_attn": jax.random.normal(ks[1], (C, 3 * C), dtype=jnp.float32) * 0.02,
        "b_attn": jnp.zeros((3 * C,), dtype=jnp.float32),
        "W_proj": jax.random.normal(ks[2], (C, C), dtype=jnp.float32) * 0.02,
        "b_proj": jnp.zeros((C,), dtype=jnp.float32),
    }
    out = kernel(**{k: np.asarray(v) for k, v in inputs.items()})
    print(out.shape, out.dtype)


# revision 31
# speedup vs baseline: 1.0593x; 1.0238x over previous
"""Causal self-attention (B=4, T=2048, C=1024, NH=16) on 8 TRN2 NeuronCores.

Sharding: tensor-parallel over heads - 2 heads per core. Each core computes
its slice of qkv (transposed layout), full causal attention for its heads,
and a partial output projection; the host sums the 8 bf16 partials and adds
b_proj.

All matmul operands are bf16 (fp32 PSUM accumulation). bf16 enables the
fast-weight-load path so LDWEIGHTS overlaps the matmul stream, and halves
DMA + DVE-copy traffic. Measured rel err stays ~1e-3, far under the 2e-2
budget.

Attention is emitted qc-major (one 512-wide q chunk at a time per batch):
for each k-chunk j the two heads' score matmuls write one [128,1024] PSUM
pair, a single fused Exp covers both heads, and pv accumulates into one
[65,512] PSUM bank per head (ones-column computes the softmax denominator).
The j-loop is software-pipelined as sc(j) / exp(j) / pv(j-1) so the PE
never sits on the ACT dependency, with qkv(b+1) / vT-transpose / proj(b-1)
units interleaved between attention units to keep the PE array dense (and
HAM-warm).

PSUM budget (8 banks): sc pair 2 + yts 2 + qkv fill 2 + proj 1 + vT 0.25.
"""

import sys

import numpy as np

try:
    import concourse.bass as bass
except ImportError:  # grading container may not have it on sys.path
    sys.path.insert(0, "/opt/trn_rl_repo")
    import concourse.bass as bass

from contextlib import ExitStack

import ml_dtypes
import concourse.mybir as mybir
import concourse.tile as tile
from concourse.bass_utils import run_bass_kernel_spmd


B, T, C, NH, HD = 4, 2048, 1024, 16, 64
N_CORES = 8
HPC = NH // N_CORES  # heads per core = 2
DPC = HPC * HD  # dims per core = 128
BT = B * T  # 8192
QCH = 512  # q-chunk
KCH = 128  # k-chunk
TCH = 512  # token chunk for qkv
NKC = C // 128  # 8 contraction chunks for qkv
NTC = T // TCH  # 4 token chunks per batch
NQC = T // QCH  # 4 q-chunks per batch
NVC = T // KCH  # 16 k/v chunks per batch
F32 = mybir.dt.float32
BF16 = mybir.dt.bfloat16
AF = mybir.ActivationFunctionType


def _split_multi_waits(nc):
    """Walrus in this container accepts only ONE sync wait per instruction.
    Hoist extra waits onto same-engine NoOps inserted just before."""
    n = 0
    for f in nc.m.functions:
        for b in f.blocks:
            insts = b.instructions
            if not any(
                i.sync_info is not None
                and i.sync_info.on_wait
                and len(i.sync_info.on_wait) > 1
                for i in insts
            ):
                continue
            new = []
            for ins in insts:
                si = ins.sync_info
                if si is not None and si.on_wait and len(si.on_wait) > 1:
                    waits = list(si.on_wait)
                    for w in waits[:-1]:
                        nop = mybir.InstNoOp(
                            name=f"{ins.name}-ws{n}", ins=[], outs=[]
                        )
                        nop.engine = ins.engine
                        nop.bass_nofuse = True
                        nop.sync_info = mybir.SyncInfo(on_wait=[w], on_update=[])
                        if ins.debug is not None:
                            nop.debug = ins.debug
                        new.append(nop)
                        n += 1
                    ins.sync_info = mybir.SyncInfo(
                        on_wait=[waits[-1]], on_update=list(si.on_update or [])
                    )
                new.append(ins)
            b.instructions = new
    return n


def build_kernel():
    nc = bass.Bass("TRN2", target_bir_lowering=False, debug=False, num_devices=N_CORES)
    xT_d = nc.dram_tensor("xT", [C, BT], BF16, kind="ExternalInput")
    wc_d = nc.dram_tensor("wc", [C, 3 * DPC], BF16, kind="ExternalInput")
    bc_d = nc.dram_tensor("bc", [3, DPC, 1], F32, kind="ExternalInput")
    wp_d = nc.dram_tensor("wp", [DPC, C], BF16, kind="ExternalInput")
    out_d = nc.dram_tensor("out", [BT, C], BF16, kind="ExternalOutput")
    # scratch for the softmax-reciprocal row: bouncing through DRAM lets
    # the broadcast read use a stride-0 row (illegal on an SBUF source)
    rc_d = nc.dram_tensor("rc_scratch", [B * NQC, 2 * QCH], BF16, kind="Internal")

    with tile.TileContext(nc) as tc, ExitStack() as ctx:
        consts = ctx.enter_context(tc.tile_pool(name="consts", bufs=1))
        xpool = ctx.enter_context(tc.tile_pool(name="x", bufs=16))
        qkvp = ctx.enter_context(tc.tile_pool(name="qkv", bufs=2))
        vexp = ctx.enter_context(tc.tile_pool(name="vext", bufs=2))
        ytp = ctx.enter_context(tc.tile_pool(name="yt", bufs=2))
        expp = ctx.enter_context(tc.tile_pool(name="expt", bufs=6))
        smallp = ctx.enter_context(tc.tile_pool(name="small", bufs=3))
        outp = ctx.enter_context(tc.tile_pool(name="outt", bufs=4))
        # PSUM: 8 banks total. sc 2 + yt 2 + qk 2 + po 1 + pt 0.25
        ps_sc = ctx.enter_context(tc.tile_pool(name="ps_sc", bufs=2, space="PSUM"))
        ps_yt = ctx.enter_context(tc.tile_pool(name="ps_yt", bufs=2, space="PSUM"))
        ps_qk = ctx.enter_context(tc.tile_pool(name="ps_qk", bufs=1, space="PSUM"))
        ps_po = ctx.enter_context(tc.tile_pool(name="ps_po", bufs=1, space="PSUM"))

        # [128,128] bf16 identity for PE transposes
        ident = consts.tile([128, 128], BF16)
        nc.gpsimd.memset(ident, 0.0)
        nc.gpsimd.affine_select(
            out=ident,
            in_=ident,
            compare_op=mybir.AluOpType.not_equal,
            fill=1.0,
            base=0,
            pattern=[[-1, 128]],
            channel_multiplier=1,
        )

        # weights: wc [1024, 384] -> [128, 8, 384] (kc chunks on free dim)
        w_sb = consts.tile([128, 8, 3 * DPC], BF16)
        nc.sync.dma_start(
            out=w_sb, in_=wc_d.ap().rearrange("(kc p) c -> p kc c", p=128)
        )
        wp_sb = consts.tile([128, C], BF16)
        nc.sync.dma_start(out=wp_sb, in_=wp_d.ap())
        bc_sb = consts.tile([128, 3], F32)
        nc.sync.dma_start(out=bc_sb, in_=bc_d.ap().rearrange("g p one -> p (g one)"))

        # -------- unit-based emission with explicit cross-phase interleave.
        state = {}

        def qkv_units(b):
            t0 = b * T
            st = state.setdefault(b, {})
            units = []

            def alloc(b=b, st=st):
                st["qT"] = qkvp.tile([128, T], BF16, name=f"qT_{b}", tag="qT")
                st["kT"] = qkvp.tile([128, T], BF16, name=f"kT_{b}", tag="kT")
                st["vT"] = qkvp.tile([128, T], BF16, name=f"vT_{b}", tag="vT")
                st["xts"] = {}

            units.append(alloc)
            for tcb in range(NTC):

                def dma_u(tcb=tcb, st=st, t0=t0, b=b):
                    xts = []
                    for kc in range(NKC):
                        xt = xpool.tile(
                            [128, TCH], BF16, name=f"xt_{b}_{tcb}_{kc}", tag="xt"
                        )
                        nc.sync.dma_start(
                            out=xt,
                            in_=xT_d.ap()[
                                kc * 128 : (kc + 1) * 128,
                                t0 + tcb * TCH : t0 + (tcb + 1) * TCH,
                            ],
                        )
                        xts.append(xt)
                    st["xts"][tcb] = xts

                units.append(dma_u)
                for g in range(3):
                    # split each accumulation group in two emission units so
                    # fills interleave at finer grain (same psum tile).
                    def mm_u1(tcb=tcb, g=g, st=st, b=b):
                        ps = ps_qk.tile(
                            [128, TCH], F32, name=f"qkvps_{b}_{tcb}_{g}", tag="qk"
                        )
                        st["qkv_ps"] = ps
                        for kc in range(4):
                            nc.tensor.matmul(
                                ps,
                                w_sb[:, kc, g * 128 : (g + 1) * 128],
                                st["xts"][tcb][kc],
                                start=(kc == 0),
                                stop=False,
                            )

                    def mm_u2(tcb=tcb, g=g, st=st, b=b):
                        ps = st["qkv_ps"]
                        dest = [st["qT"], st["kT"], st["vT"]]
                        for kc in range(4, NKC):
                            nc.tensor.matmul(
                                ps,
                                w_sb[:, kc, g * 128 : (g + 1) * 128],
                                st["xts"][tcb][kc],
                                start=False,
                                stop=(kc == NKC - 1),
                            )
                        # psum -> sbuf bf16 with bias add on DVE
                        nc.vector.tensor_scalar_add(
                            dest[g][:, tcb * TCH : (tcb + 1) * TCH],
                            ps,
                            bc_sb[:, g : g + 1],
                        )

                    units.append(mm_u1)
                    units.append(mm_u2)
            return units

        def vt_units(b):
            st = state.setdefault(b, {})
            units = []

            def alloc(st=st, b=b):
                st["vex"] = vexp.tile(
                    [128, NVC, HPC, 65], BF16, name=f"vex_{b}", tag="vex"
                )
                nc.vector.memset(st["vex"][:, :, :, 64:65], 1.0)

            units.append(alloc)
            for j in range(NVC):

                def tr_u(j=j, st=st, b=b):
                    # transpose both heads' [128d, 128t] chunk in one PE shot
                    # (sharing the proj psum ring), then one fused DVE copy
                    ptf = ps_po.tile([128, 512], F32, name=f"vtps_{b}_{j}", tag="po")
                    pt = ptf[:, 0:64].bitcast(BF16)  # [128, 128] bf16 view
                    nc.tensor.transpose(
                        pt, st["vT"][:, j * 128 : (j + 1) * 128], ident
                    )
                    nc.vector.tensor_copy(
                        st["vex"][:, j, :, 0:64],
                        pt.rearrange("p (h d) -> p h d", h=HPC),
                    )

                units.append(tr_u)
            return units

        def attn_units(b, proj_tail=None):
            """proj_tail: optional per-qc list of extra units (e.g. proj of
            this batch's finished q-chunks) appended after each qc's norm."""
            st = state[b]
            units = []

            def alloc_yt(st=st, b=b):
                st["yT"] = ytp.tile([128, T], BF16, name=f"yT_{b}", tag="yT")

            units.append(alloc_yt)
            pend_early = []  # bank-draining norm units of the previous qc
            pend_late = []  # rest of the previous qc's norm chain + proj tail
            for qc in range(NQC):
                nj = 4 * (qc + 1)  # k-chunks for this q chunk

                def alloc_chains(qc=qc, st=st, b=b):
                    st[("yts", qc)] = [
                        ps_yt.tile(
                            [65, QCH], F32, name=f"yt_{b}_{qc}_{h}", tag="yt"
                        )
                        for h in range(HPC)
                    ]

                def sc_u(j, qc=qc, st=st, b=b):
                    q0 = qc * QCH
                    k0 = j * KCH
                    qlo = max(0, k0 - q0)
                    sc = ps_sc.tile(
                        [128, 2 * QCH], F32, name=f"sc_{b}_{qc}_{j}", tag="sc"
                    )
                    st[("sc", j)] = (sc, qlo)
                    # 4-way 64x64 array tiling: (head h -> row group 64h,
                    # key-half kh -> col group 64kh) are pairwise-disjoint
                    # sub-rectangles of the PE array, so all four matmuls
                    # stream concurrently.
                    for h in range(HPC):
                        for kh in range(2):
                            nc.tensor.matmul(
                                sc[64 * kh : 64 * kh + 64, h * QCH + qlo : (h + 1) * QCH],
                                st["kT"][
                                    64 * h : 64 * h + 64,
                                    k0 + 64 * kh : k0 + 64 * kh + 64,
                                ],
                                st["qT"][64 * h : 64 * h + 64, q0 + qlo : q0 + QCH],
                                start=True,
                                stop=True,
                            )

                def exp_u(j, qc=qc, st=st, b=b):
                    sc, qlo = st[("sc", j)]
                    ex = expp.tile(
                        [128, 2 * QCH], BF16, name=f"ex_{b}_{qc}_{j}", tag="ex"
                    )
                    st[("ex", j)] = (ex, qlo)
                    if qlo == 0:
                        # one fused exp across both heads
                        nc.scalar.activation(
                            ex[:, 0 : 2 * QCH], sc[:, 0 : 2 * QCH], AF.Exp,
                            scale=0.125,
                        )
                    else:
                        # diagonal block: skip the stale gap between the two
                        # heads' valid column ranges
                        for h in range(HPC):
                            nc.scalar.activation(
                                ex[:, h * QCH + qlo : (h + 1) * QCH],
                                sc[:, h * QCH + qlo : (h + 1) * QCH],
                                AF.Exp,
                                scale=0.125,
                            )
                    if j * KCH >= qc * QCH:
                        # diagonal block: zero where k > q, per head
                        for h in range(HPC):
                            nc.gpsimd.affine_select(
                                out=ex[:, h * QCH + qlo : h * QCH + qlo + 128],
                                in_=ex[:, h * QCH + qlo : h * QCH + qlo + 128],
                                compare_op=mybir.AluOpType.is_ge,
                                fill=0.0,
                                base=0,
                                pattern=[[1, 128]],
                                channel_multiplier=-1,
                            )

                def pv_u(j, qc=qc, nj=nj, st=st, b=b):
                    ex, qlo = st[("ex", j)]
                    for h in range(HPC):
                        nc.tensor.matmul(
                            st[("yts", qc)][h][:, qlo:QCH],
                            st["vex"][:, j, h, :],
                            ex[:, h * QCH + qlo : (h + 1) * QCH],
                            start=(j == 0),
                            stop=(j == nj - 1),
                        )

                def norm_u1(qc=qc, st=st, b=b):
                    # drain the yts banks as fast as possible: copy the
                    # unnormalized y to SBUF and Ln the sums row; the
                    # reciprocal/broadcast/multiply happen later, off the
                    # next q-chunk's critical path. Ln+Exp share one ACT
                    # table set; Ln from PSUM skips a DVE gather.
                    ysb = smallp.tile(
                        [64, 2 * QCH], BF16, name=f"ysb_{b}_{qc}", tag="ysb"
                    )
                    st[("ysb", qc)] = ysb
                    sums = smallp.tile([1, 2 * QCH], F32, name=f"s_{b}_{qc}", tag="s")
                    st[("sums", qc)] = sums
                    for h in range(HPC):
                        nc.vector.tensor_copy(
                            ysb[:, h * QCH : (h + 1) * QCH],
                            st[("yts", qc)][h][0:64, :],
                        )
                        nc.scalar.activation(
                            sums[:, h * QCH : (h + 1) * QCH],
                            st[("yts", qc)][h][64:65, :],
                            AF.Ln,
                        )

                def norm_u2(qc=qc, st=st, b=b):
                    recip = smallp.tile(
                        [1, 2 * QCH], BF16, name=f"rc_{b}_{qc}", tag="rc"
                    )
                    nc.scalar.activation(recip, st[("sums", qc)], AF.Exp, scale=-1.0)
                    nc.sync.dma_start(
                        out=rc_d.ap()[b * NQC + qc : b * NQC + qc + 1, :], in_=recip
                    )

                def norm_u3(h, qc=qc, st=st, b=b):
                    # broadcast recip row to 64 partitions on the DMA engine
                    bch = smallp.tile(
                        [64, QCH], BF16, name=f"bc_{b}_{qc}_{h}", tag=f"bc{h}"
                    )
                    nc.sync.dma_start(
                        out=bch,
                        in_=rc_d.ap()[
                            b * NQC + qc : b * NQC + qc + 1,
                            h * QCH : (h + 1) * QCH,
                        ].to_broadcast((64, QCH)),
                    )
                    nc.vector.tensor_mul(
                        st["yT"][64 * h : 64 * h + 64, qc * QCH : (qc + 1) * QCH],
                        st[("ysb", qc)][:, h * QCH : (h + 1) * QCH],
                        bch,
                    )

                # software pipeline: sc(j), exp(j), pv(j-1). The previous
                # qc's bank-draining norm_u1 (and alloc_chains, whose ring
                # slots are the previous qc's yts) go right after sc(0)/
                # exp(0); the rest of its norm chain and any ready proj-tail
                # units go at j==2, far off the pv(0) critical path.
                for j in range(nj):
                    units.append(lambda j=j, f=sc_u: f(j))
                    units.append(lambda j=j, f=exp_u: f(j))
                    if j == 0:
                        units.extend(pend_early)
                        pend_early = []
                        units.append(alloc_chains)
                    if j == 2:
                        units.extend(pend_late)
                        pend_late = []
                    if j > 0:
                        units.append(lambda j=j, f=pv_u: f(j - 1))
                units.append(lambda nj=nj, f=pv_u: f(nj - 1))

                pend_early = [norm_u1]
                pend_late = [
                    norm_u2,
                    lambda f=norm_u3: f(0),
                    lambda f=norm_u3: f(1),
                ]
                if proj_tail is not None:
                    pend_late = pend_late + proj_tail[qc]
            units.extend(pend_early)
            units.extend(pend_late)
            return units

        def proj_units(b, extra_pool=False, by_qc=False):
            st = state[b]
            t0 = b * T
            units = []
            for tcb in range(T // 128):
                for g in range(2):

                    def p_u(tcb=tcb, g=g, st=st, t0=t0, b=b):
                        pool = ps_qk if (extra_pool and g == 1) else ps_po
                        tag = "qk" if (extra_pool and g == 1) else "po"
                        ps = pool.tile(
                            [128, 512], F32, name=f"pps_{b}_{tcb}_{g}", tag=tag
                        )
                        nc.tensor.matmul(
                            ps,
                            st["yT"][:, tcb * 128 : (tcb + 1) * 128],
                            wp_sb[:, g * 512 : (g + 1) * 512],
                            start=True,
                            stop=True,
                        )
                        ot = outp.tile(
                            [128, 512], BF16, name=f"ot_{b}_{tcb}_{g}", tag="ot"
                        )
                        nc.vector.tensor_copy(ot, ps)
                        nc.sync.dma_start(
                            out=out_d.ap()[
                                t0 + tcb * 128 : t0 + (tcb + 1) * 128,
                                g * 512 : (g + 1) * 512,
                            ],
                            in_=ot,
                        )

                    units.append(p_u)
            if by_qc:  # group by the q-chunk whose yT cols they read
                return [units[8 * qc : 8 * qc + 8] for qc in range(NQC)]
            return units

        def interleave(main, fill):
            """emit main units with fill units spread evenly between them"""
            out = []
            nf, nm = len(fill), len(main)
            fi = 0
            for mi, m in enumerate(main):
                out.append(m)
                want = (mi + 1) * nf // nm
                while fi < want:
                    out.append(fill[fi])
                    fi += 1
            out.extend(fill[fi:])
            return out

        for u in qkv_units(0) + vt_units(0):
            u()
        for b in range(B):
            tail = (
                proj_units(b, extra_pool=True, by_qc=True) if b == B - 1 else None
            )
            main = attn_units(b, proj_tail=tail)
            fill = []
            if b + 1 < B:
                fill += qkv_units(b + 1) + vt_units(b + 1)
            if b >= 1:
                fill += proj_units(b - 1, extra_pool=(b - 1 == 2))
            for u in interleave(main, fill):
                u()

    _split_multi_waits(nc)
    return nc


_NC_CACHE = None


def _get_nc():
    global _NC_CACHE
    if _NC_CACHE is None:
        _NC_CACHE = build_kernel()
    return _NC_CACHE


def kernel_with_results(x, W_attn, b_attn, W_proj, b_proj, trace=False):
    bf = ml_dtypes.bfloat16
    x = np.asarray(x, dtype=np.float32)
    W_attn = np.asarray(W_attn, dtype=np.float32)
    b_attn = np.asarray(b_attn, dtype=np.float32)
    W_proj = np.asarray(W_proj, dtype=np.float32)
    b_proj = np.asarray(b_proj, dtype=np.float32)

    xT = np.ascontiguousarray(x.reshape(BT, C).T).astype(bf)  # [C, BT]
    in_maps = []
    for c in range(N_CORES):
        lo = c * DPC
        wc = np.ascontiguousarray(
            np.concatenate(
                [
                    W_attn[:, lo : lo + DPC],
                    W_attn[:, C + lo : C + lo + DPC],
                    W_attn[:, 2 * C + lo : 2 * C + lo + DPC],
                ],
                axis=1,
            )
        ).astype(bf)
        bc = np.ascontiguousarray(
            np.stack(
                [
                    b_attn[lo : lo + DPC],
                    b_attn[C + lo : C + lo + DPC],
                    b_attn[2 * C + lo : 2 * C + lo + DPC],
                ]
            ).reshape(3, DPC, 1)
        )
        wp = np.ascontiguousarray(W_proj[lo : lo + DPC, :]).astype(bf)
        in_maps.append({"xT": xT, "wc": wc, "bc": bc, "wp": wp})

    nc = _get_nc()
    res = run_bass_kernel_spmd(
        nc, in_maps, core_ids=list(range(N_CORES)), trace=trace
    )
    acc = np.zeros((BT, C), dtype=np.float32)
    for c in range(N_CORES):
        acc += res.results[c]["out"].astype(np.float32)
    out = acc + b_proj
    return out.reshape(B, T, C), res


def kernel(x, W_attn, b_attn, W_proj, b_proj):
    out, _ = kernel_with_results(x, W_attn, b_attn, W_proj, b_proj)
    return out


if __name__ == "__main__":
    import jax

    key = jax.random.key(0)
    ks = jax.random.split(key, 5)
    import jax.numpy as jnp

    inputs = {
        "x": jax.random.normal(ks[0], (B, T, C), dtype=jnp.float32),
        "W_attn": jax.random.normal(ks[1], (C, 3 * C), dtype=jnp.float32) * 0.02,
        "b_attn": jnp.zeros((3 * C,), dtype=jnp.float32),
        "W_proj": jax.random.normal(ks[2], (C, C), dtype=jnp.float32) * 0.02,
        "b_proj": jnp.zeros((C,), dtype=jnp.float32),
    }
    out = kernel(**{k: np.asarray(v) for k, v in inputs.items()})
    print(out.shape, out.dtype)
